# revision 42
# baseline (speedup 1.0000x reference)
"""Trainium2 Bass kernel for nn_MixtureCogrammar.

Computation (reference):
    attn  = softmax(morphosyn @ W_affix)                    [B, V]
    affix = attn @ affix_vocab.reshape(V, D*N)              [B, D, N]
    wC    = cumsum_n( sum_{ijk} a_i b_j f_k softmax(pivot_logits[i,j,:,k,:]) )
    out   = stem + wC * (affix - stem)

Distribution: D sharded over 8 cores (DLOC=32). Attention replicated
(cheap), pivots/wC batch-sharded + AllGather, stem/vocab/out D-sharded.

Per-core program:
  - pivot softmax/cumsum + wC AllGather issued FIRST so the gather is
    never on the critical path of the main loop (the collective still
    waits out the inter-core start skew, absorbed by the runahead below)
  - attn computed per 128-batch chunk in bf16 (bf16 morphosyn/W_affix
    matmul), quantized to fp8e4 scaled by 64 (values land in fp8-normal
    range), PE-transposed to a resident [V, B] fp8 lhsT
  - main matmul: plain fp8 matmuls ordered [vc outer, t inner] so
    consecutive instructions share the stationary tile; a post-compile
    BIR pass (_dedup_ldweights) drops the redundant LDWEIGHTS reloads
    bass emits per matmul (384 of 560), which otherwise serialize on
    the PE and make it the pacing engine
  - ScalarE drains PSUM with a 1/64 scale to bf16; DVE (the pacing
    engine, ~128us busy) runs sub/mul/add on all-bf16 SBUF operands in
    its 2x mode, at full [128, 4096] granularity; delta/stem pools are
    deep enough to run ~7 iterations ahead while waiting for wC (the
    attn scratch pool is released and prod runs single-buffered — free
    on the in-order DVE queue — to make the last pair fit SBUF)
  - measured non-starters kept out: fp8 DoubleRow (no per-row gain on
    this hw), GpSimd tensor ops co-running with DVE (4x DVE slowdowns),
    splitting DMAs across the Act/Pool queues, per-(i,j) pivot
    pipelines, and tile_wait_until reordering (all regressed)
"""

import os
import sys

import numpy as np

for _p in ("/opt/trn_rl_repo",):
    if os.path.isdir(_p) and _p not in sys.path:
        sys.path.append(_p)

import concourse.bass as bass  # noqa: E402
import concourse.tile as tile  # noqa: E402
from concourse import bacc, mybir  # noqa: E402
from concourse.bass import ts  # noqa: E402
from concourse.bass_utils import run_bass_kernel_spmd  # noqa: E402
from concourse.masks import make_identity  # noqa: E402

import ml_dtypes  # noqa: E402

import json as _json  # noqa: E402


def _dedup_ldweights(nc):
    """Drop Ldweights that reload the exact weights already resident.

    Bass legalization emits an InstLdweights before every InstMatmult even
    when consecutive matmuls share the same stationary tile; each reload
    serializes ~130-190ns on the PE, which for this kernel's 512 matmuls
    makes the PE the pacing engine. The PE array keeps its weights between
    matmuls, so a reload of the identical AP is a no-op — remove it when
    it carries no semaphore waits/updates (sync stays intact). The result
    is spliced in via a to_json_bytes override; correctness is verified
    against the reference on hardware.
    """
    bir = _json.loads(nc.to_json_bytes())
    dropped = 0
    for f in bir.get("functions", []):
        for b in f.get("blocks", []):
            cur = None
            out = []
            for i in b.get("instructions", []):
                op = i.get("opcode")
                if op == "Ldweights":
                    sig = _json.dumps(
                        [i.get("ins"), i.get("perf_mode"), i.get("is_transpose")],
                        sort_keys=True)
                    si = i.get("sync_info") or {}
                    if (sig == cur and not (si.get("on_wait") or [])
                            and not (si.get("on_update") or [])):
                        dropped += 1
                        continue
                    cur = sig
                elif op != "Matmult":
                    pass  # non-PE-datapath ops don't clobber PE weights
                out.append(i)
            b["instructions"] = out
    data = _json.dumps(bir).encode()
    nc.to_json_bytes = lambda: data
    return dropped

B, D, N, DM, V = 1024, 256, 256, 128, 512
NCORES = 8
DLOC = D // NCORES          # 32 d-values per core
BCH = B // 128              # 8 batch chunks
DN = DLOC * N               # 8192 free elems per core
HALF = DN // 2              # 4096 per round
DHALF = DLOC // 2           # 16 d-values per round
PSW = 2048                  # one psum tile = 4 banks
NH = HALF // PSW            # 2 psum tiles per (chunk, round)
ASCALE = 64.0               # attn quantization scale into fp8 range

F32 = mybir.dt.float32
BF16 = mybir.dt.bfloat16
F8 = mybir.dt.float8e4
EXP = mybir.ActivationFunctionType.Exp
ALU = mybir.AluOpType

LAST_RESULT = None

_CACHE = {}


def _build():
    key = 0
    if key in _CACHE:
        return _CACHE[key]

    nc = bacc.Bacc("TRN2", target_bir_lowering=False, debug=False,
                   num_devices=NCORES)

    stem_d = nc.dram_tensor("stem", [B, DLOC, N], BF16, kind="ExternalInput").ap()
    vocab_d = nc.dram_tensor("vocab", [V, DLOC, N], F8, kind="ExternalInput").ap()
    mor_d = nc.dram_tensor("morpho", [B, DM], BF16, kind="ExternalInput").ap()
    waff_d = nc.dram_tensor("waffix", [DM, V], BF16, kind="ExternalInput").ap()
    pv_d = nc.dram_tensor("pivot", [2, 2, 128, 5, N], BF16, kind="ExternalInput").ap()
    abf_d = nc.dram_tensor("abf", [1, 9], F32, kind="ExternalInput").ap()
    out_d = nc.dram_tensor("out", [B, DLOC, N], BF16, kind="ExternalOutput").ap()

    from contextlib import ExitStack

    with tile.TileContext(nc) as tc, ExitStack() as ctx:
        const = ctx.enter_context(tc.tile_pool(name="const", bufs=1))

        identf = const.tile([128, 128], F32)
        make_identity(nc, identf[:, :])
        identb = const.tile([128, 128], BF16)
        nc.scalar.copy(identb[:, :], identf[:, :])

        attnT = const.tile([128, 4, B], F8)        # [v_part, vc, b], 64*attn
        wc_sb = const.tile([128, BCH, N], BF16)    # [b_part, cb, n]
        w_bcast = const.tile([128, 20], F32)
        wsb = const.tile([128, V], BF16)           # W_affix resident [dm, v]
        mor_all = const.tile([128, BCH, DM], BF16)

        # ---------- phase P: mixture weights + pivots + AllGather ----------
        small = ctx.enter_context(tc.tile_pool(name="small", bufs=1))
        pvp = tc.alloc_tile_pool(name="pv", bufs=1)
        pv = pvp.tile([128, 4, 5, N], BF16)
        abf = small.tile([1, 9], F32)
        nc.sync.dma_start(abf[0:1, :], abf_d[:, :])
        for ij in range(4):
            i, j = divmod(ij, 2)
            nc.sync.dma_start(pv[:, ij, :, :], pv_d[i, j, :, :, :])
        pvE = pvp.tile([128, 4, 5, N], BF16)
        sP = pvp.tile([128, 4, 5], F32)
        # exp+rowsum pipelined per (i,j) chunk so compute starts on the
        # first pivot DMA rather than the last
        for ij in range(4):
            nc.scalar.activation(
                pvE[:, ij, :, :].rearrange("p k n -> p (k n)"),
                pv[:, ij, :, :].rearrange("p k n -> p (k n)"), EXP,
            )
            nc.vector.reduce_sum(sP[:, ij, :], pvE[:, ij, :, :],
                                 axis=mybir.AxisListType.X)

        # mixture weights w20[g] = a_i * b_j * f_k (g = (i*2+j)*5+k)
        eabf = small.tile([1, 9], F32)
        sums3 = small.tile([1, 3], F32)
        nc.scalar.activation(eabf[0:1, 0:2], abf[0:1, 0:2], EXP, accum_out=sums3[0:1, 0:1])
        nc.scalar.activation(eabf[0:1, 2:4], abf[0:1, 2:4], EXP, accum_out=sums3[0:1, 1:2])
        nc.scalar.activation(eabf[0:1, 4:9], abf[0:1, 4:9], EXP, accum_out=sums3[0:1, 2:3])
        rsum = small.tile([1, 3], F32)
        nc.vector.reciprocal(rsum[0:1, :], sums3[0:1, :])
        t4 = small.tile([1, 4], F32)
        nc.vector.tensor_mul(
            t4[0:1, :].rearrange("p (i j) -> p i j", i=2),
            eabf[0:1, 0:2].rearrange("p (i j) -> p i j", j=1).to_broadcast((1, 2, 2)),
            eabf[0:1, 2:4].rearrange("p (i j) -> p i j", i=1).to_broadcast((1, 2, 2)),
        )
        t20 = small.tile([1, 20], F32)
        nc.vector.tensor_mul(
            t20[0:1, :].rearrange("p (g k) -> p g k", g=4),
            t4[0:1, :].rearrange("p (g k) -> p g k", k=1).to_broadcast((1, 4, 5)),
            eabf[0:1, 4:9].rearrange("p (g k) -> p g k", g=1).to_broadcast((1, 4, 5)),
        )
        rr = small.tile([1, 1], F32)
        nc.vector.tensor_mul(rr[0:1, :], rsum[0:1, 0:1], rsum[0:1, 1:2])
        rrr = small.tile([1, 1], F32)
        nc.vector.tensor_mul(rrr[0:1, :], rr[0:1, :], rsum[0:1, 2:3])
        w20 = small.tile([1, 20], F32)
        nc.vector.tensor_scalar_mul(w20[0:1, :], t20[0:1, :], rrr[0:1, 0:1])
        nc.gpsimd.partition_broadcast(w_bcast[:, :], w20[0:1, :])

        # weighted sum over the 20 groups, then cumsum. Few big DVE ops
        # beat a per-chunk pipeline here: this chain sits at the head of
        # the DVE queue, and many small sem-gated ops there delay the
        # main-loop runahead subs queued behind them.
        rP = pvp.tile([128, 20], F32)
        nc.vector.reciprocal(rP[:, :], sP[:, :, :].rearrange("p i k -> p (i k)"))
        rPw = pvp.tile([128, 20], BF16)
        nc.vector.tensor_mul(rPw[:, :], rP[:, :], w_bcast[:, :])
        pvS = pvp.tile([128, 20, N], BF16)
        nc.vector.tensor_mul(
            pvS[:, :, :],
            pvE[:, :, :, :].rearrange("p i k n -> p (i k) n"),
            rPw[:, :].rearrange("p (g o) -> p g o", o=1).to_broadcast((128, 20, N)),
        )
        t10 = pvp.tile([128, 10, N], BF16)
        nc.vector.tensor_add(t10[:, :, :], pvS[:, 0:10, :], pvS[:, 10:20, :])
        t5 = pvp.tile([128, 5, N], BF16)
        nc.vector.tensor_add(t5[:, :, :], t10[:, 0:5, :], t10[:, 5:10, :])
        t2 = pvp.tile([128, 2, N], BF16)
        nc.vector.tensor_add(t2[:, :, :], t5[:, 0:2, :], t5[:, 2:4, :])
        acc1 = pvp.tile([128, N], BF16)
        nc.vector.tensor_add(acc1[:, :], t2[:, 0, :], t2[:, 1, :])
        accf = pvp.tile([128, N], F32)
        nc.vector.tensor_add(accf[:, :], acc1[:, :], t5[:, 4, :])
        wCl = pvp.tile([128, N], BF16)
        nc.vector.tensor_tensor_scan(
            wCl[:, :], data0=accf[:, :], data1=accf[:, :], initial=0.0,
            op0=ALU.add, op1=ALU.bypass,
        )
        dram = ctx.enter_context(tc.tile_pool(name="dram", bufs=1, space="DRAM"))
        wc_in = dram.tile([128, N], BF16)
        wc_out = nc.dram_tensor("wc_gath", [B, N], BF16, addr_space="Shared").ap()
        nc.sync.dma_start(wc_in[:, :], wCl[:, :])
        nc.gpsimd.collective_compute(
            "AllGather", ALU.bypass,
            replica_groups=[list(range(NCORES))],
            ins=[wc_in[:, :].opt()], outs=[wc_out[:, :].opt()],
        )
        nc.sync.dma_start(
            wc_sb[:, :, :],
            wc_out[:, :].rearrange("(c p) n -> p c n", p=128),
        )
        pvp.release()

        # ---------- phase A: attention -> fp8 attnT ----------
        nc.sync.dma_start(wsb[:, :], waff_d[:, :])
        nc.sync.dma_start(
            mor_all[:, :, :],
            mor_d[:, :].rearrange("(c p) m -> p c m", p=128),
        )
        bp = tc.alloc_tile_pool(name="attn", bufs=2)
        psB = tc.alloc_tile_pool(name="psB", bufs=2, space="PSUM")
        psT = tc.alloc_tile_pool(name="psT", bufs=2, space="PSUM")

        for cb in range(BCH):
            morT_ps = psB.tile([128, DM], BF16, tag="morT_ps", name=f"mtp{cb}")
            nc.tensor.transpose(morT_ps[:, :], mor_all[:, cb, :], identb[:, :])
            morT = bp.tile([128, DM], BF16, tag="morT", name=f"mt{cb}")
            nc.vector.tensor_copy(morT[:, :], morT_ps[:, :])
            lg_ps = psB.tile([128, V], F32, tag="lg_ps", name=f"lgp{cb}")
            nc.tensor.matmul(lg_ps[:, :], lhsT=morT[:, :], rhs=wsb[:, :],
                             start=True, stop=True)
            E = bp.tile([128, V], BF16, tag="E", name=f"E{cb}")
            sE = bp.tile([128, 1], F32, tag="sE", name=f"sE{cb}")
            nc.scalar.activation(E[:, :], lg_ps[:, :], EXP, accum_out=sE[:, :])
            rE = bp.tile([128, 1], F32, tag="rE", name=f"rE{cb}")
            nc.vector.reciprocal(rE[:, :], sE[:, :])
            rE64 = bp.tile([128, 1], F32, tag="rE64", name=f"rE64{cb}")
            nc.vector.tensor_scalar_mul(rE64[:, :], rE[:, :], ASCALE)
            attnb = bp.tile([128, V], BF16, tag="atb", name=f"atb{cb}")
            nc.scalar.mul(attnb[:, :], E[:, :], rE64[:, 0:1])
            tpb = psT.tile([128, V], BF16, tag="tpb", name=f"tpb{cb}")
            for vc in range(4):
                nc.tensor.transpose(tpb[:, ts(vc, 128)], attnb[:, ts(vc, 128)],
                                    identb[:, :])
            nc.scalar.copy(
                attnT[:, :, ts(cb, 128)],
                tpb[:, :].rearrange("p (c b) -> p c b", c=4),
            )
        psT.release()
        psB.release()
        bp.release()

        # ---------- phase D: main loop ----------
        stp = ctx.enter_context(tc.tile_pool(name="stem", bufs=7))
        vqp = ctx.enter_context(tc.tile_pool(name="vq", bufs=2))
        afp = ctx.enter_context(tc.tile_pool(name="affx", bufs=2))
        dlp = ctx.enter_context(tc.tile_pool(name="delta", bufs=8))
        prp = ctx.enter_context(tc.tile_pool(name="prod", bufs=1))
        otp = ctx.enter_context(tc.tile_pool(name="outp", bufs=2))
        psD = ctx.enter_context(tc.tile_pool(name="psD", bufs=2, space="PSUM"))

        for r in range(2):
            vq = vqp.tile([128, 4, HALF], F8)
            for vc in range(4):
                nc.sync.dma_start(
                    vq[:, vc, :],
                    vocab_d[ts(vc, 128), ts(r, DHALF), :].rearrange("p d n -> p (d n)"),
                )
            for cb in range(BCH):
                stem_t = stp.tile([128, HALF], BF16)
                nc.sync.dma_start(
                    stem_t[:, :],
                    stem_d[ts(cb, 128), ts(r, DHALF), :].rearrange("p d n -> p (d n)"),
                )
                affx = afp.tile([128, HALF], BF16)
                for h in range(NH):
                    ps = psD.tile([128, PSW], F32)
                    # [vc outer, t inner]: 4 consecutive matmuls share the
                    # same stationary tile so the LDW dedup drops reloads
                    for vc in range(4):
                        for t in range(PSW // 512):
                            col = h * PSW + t * 512
                            nc.tensor.matmul(
                                ps[:, ts(t, 512)],
                                lhsT=attnT[:, vc, ts(cb, 128)],
                                rhs=vq[:, vc, col:col + 512],
                                start=(vc == 0), stop=(vc == 3),
                            )
                    nc.scalar.mul(affx[:, ts(h, PSW)], ps[:, :], 1.0 / ASCALE)
                delta = dlp.tile([128, HALF], BF16)
                prod = prp.tile([128, HALF], BF16)
                out_t = otp.tile([128, HALF], BF16)
                # the final two iterations run at half-tile granularity so
                # the closing drain->sub->mul->add->store chain overlaps
                # its own stores instead of serializing after the last add
                tail = (r == 1 and cb >= BCH - 2)
                for c0, c1 in ([(0, HALF)] if not tail else [(0, PSW), (PSW, HALF)]):
                    a = c1 - c0
                    nc.vector.tensor_sub(delta[:, c0:c1], affx[:, c0:c1],
                                         stem_t[:, c0:c1])
                    nc.vector.tensor_mul(
                        prod[:, c0:c1].rearrange("p (a n) -> p a n", n=N),
                        delta[:, c0:c1].rearrange("p (a n) -> p a n", n=N),
                        wc_sb[:, cb:cb + 1, :].to_broadcast((128, a // N, N)),
                    )
                    nc.vector.tensor_add(out_t[:, c0:c1], prod[:, c0:c1],
                                         stem_t[:, c0:c1])
                    nc.sync.dma_start(
                        out_d[ts(cb, 128),
                              bass.ds(r * DHALF + c0 // N, a // N), :]
                        .rearrange("p d n -> p (d n)"),
                        out_t[:, c0:c1],
                    )

    nc.compile()
    _dedup_ldweights(nc)
    _CACHE[key] = nc
    return nc


def kernel(stem_form, morphosyn, pivot_logits, W_affix, affix_vocab,
           alpha, beta, phi, max_len):
    global LAST_RESULT
    abf = np.concatenate([
        np.asarray(alpha, np.float32).ravel(),
        np.asarray(beta, np.float32).ravel(),
        np.asarray(phi, np.float32).ravel(),
    ]).reshape(1, 9)

    nc = _build()

    stem_np = np.asarray(stem_form, np.float32).astype(ml_dtypes.bfloat16)
    vocab_np = np.asarray(affix_vocab, np.float32).astype(ml_dtypes.float8_e4m3)
    mor_np = np.asarray(morphosyn, np.float32).astype(ml_dtypes.bfloat16)
    waff_np = np.asarray(W_affix, np.float32).astype(ml_dtypes.bfloat16)
    pv_np = np.asarray(pivot_logits, np.float32).astype(ml_dtypes.bfloat16)

    in_maps = []
    for c in range(NCORES):
        dlo, dhi = c * DLOC, (c + 1) * DLOC
        in_maps.append({
            "stem": np.ascontiguousarray(stem_np[:, dlo:dhi, :]),
            "vocab": np.ascontiguousarray(vocab_np[:, dlo:dhi, :]),
            "morpho": mor_np,
            "waffix": waff_np,
            "pivot": np.ascontiguousarray(pv_np[:, :, c * 128:(c + 1) * 128, :, :]),
            "abf": abf,
        })

    LAST_RESULT = run_bass_kernel_spmd(nc, in_maps, core_ids=list(range(NCORES)))
    outs = [LAST_RESULT.results[c]["out"] for c in range(NCORES)]
    out = np.concatenate([o.astype(np.float32) for o in outs], axis=1)
    return np.ascontiguousarray(out)


# revision 43
# speedup vs baseline: 1.0515x; 1.0515x over previous
"""Trainium2 Bass kernel for nn_MixtureCogrammar.

Computation (reference):
    attn  = softmax(morphosyn @ W_affix)                    [B, V]
    affix = attn @ affix_vocab.reshape(V, D*N)              [B, D, N]
    wC    = cumsum_n( sum_{ijk} a_i b_j f_k softmax(pivot_logits[i,j,:,k,:]) )
    out   = stem + wC * (affix - stem)

Distribution: D sharded over 8 cores (DLOC=32). Attention replicated
(cheap), pivots/wC batch-sharded + AllGather, stem/vocab/out D-sharded.

Per-core program:
  - pivot softmax/cumsum + wC AllGather issued FIRST so the gather is
    never on the critical path of the main loop (the collective still
    waits out the inter-core start skew, absorbed by the runahead below)
  - attn computed per 128-batch chunk in bf16 (bf16 morphosyn/W_affix
    matmul), quantized to fp8e4 scaled by 64 (values land in fp8-normal
    range), PE-transposed to a resident [V, B] fp8 lhsT
  - main matmul: plain fp8 matmuls ordered [vc outer, t inner] so
    consecutive instructions share the stationary tile; a post-compile
    BIR pass (_dedup_ldweights) drops the redundant LDWEIGHTS reloads
    bass emits per matmul (384 of 560), which otherwise serialize on
    the PE and make it the pacing engine
  - ScalarE drains PSUM with a 1/64 scale to bf16; DVE (the pacing
    engine, ~128us busy) runs sub/mul/add on all-bf16 SBUF operands in
    its 2x mode, at full [128, 4096] granularity; delta/stem pools are
    deep enough to run ~7 iterations ahead while waiting for wC (the
    attn scratch pool is released and prod runs single-buffered — free
    on the in-order DVE queue — to make the last pair fit SBUF)
  - measured non-starters kept out: fp8 DoubleRow (no per-row gain on
    this hw), GpSimd tensor ops co-running with DVE (4x DVE slowdowns),
    splitting DMAs across the Act/Pool queues, per-(i,j) pivot
    pipelines, and tile_wait_until reordering (all regressed)
"""

import os
import sys

import numpy as np

for _p in ("/opt/trn_rl_repo",):
    if os.path.isdir(_p) and _p not in sys.path:
        sys.path.append(_p)

import concourse.bass as bass  # noqa: E402
import concourse.tile as tile  # noqa: E402
from concourse import bacc, mybir  # noqa: E402
from concourse.bass import ts  # noqa: E402
from concourse.bass_utils import run_bass_kernel_spmd  # noqa: E402
from concourse.masks import make_identity  # noqa: E402

import ml_dtypes  # noqa: E402

import json as _json  # noqa: E402


def _dedup_ldweights(nc):
    """Drop Ldweights that reload the exact weights already resident.

    Bass legalization emits an InstLdweights before every InstMatmult even
    when consecutive matmuls share the same stationary tile; each reload
    serializes ~130-190ns on the PE, which for this kernel's 512 matmuls
    makes the PE the pacing engine. The PE array keeps its weights between
    matmuls, so a reload of the identical AP is a no-op — remove it when
    it carries no semaphore waits/updates (sync stays intact). The result
    is spliced in via a to_json_bytes override; correctness is verified
    against the reference on hardware.
    """
    bir = _json.loads(nc.to_json_bytes())
    dropped = 0
    for f in bir.get("functions", []):
        for b in f.get("blocks", []):
            cur = None
            out = []
            for i in b.get("instructions", []):
                op = i.get("opcode")
                if op == "Ldweights":
                    sig = _json.dumps(
                        [i.get("ins"), i.get("perf_mode"), i.get("is_transpose")],
                        sort_keys=True)
                    si = i.get("sync_info") or {}
                    if (sig == cur and not (si.get("on_wait") or [])
                            and not (si.get("on_update") or [])):
                        dropped += 1
                        continue
                    cur = sig
                elif op != "Matmult":
                    pass  # non-PE-datapath ops don't clobber PE weights
                out.append(i)
            b["instructions"] = out
    data = _json.dumps(bir).encode()
    nc.to_json_bytes = lambda: data
    return dropped

B, D, N, DM, V = 1024, 256, 256, 128, 512
NCORES = 8
DLOC = D // NCORES          # 32 d-values per core
BCH = B // 128              # 8 batch chunks
DN = DLOC * N               # 8192 free elems per core
HALF = DN // 2              # 4096 per round
DHALF = DLOC // 2           # 16 d-values per round
PSW = 2048                  # one psum tile = 4 banks
NH = HALF // PSW            # 2 psum tiles per (chunk, round)
ASCALE = 64.0               # attn quantization scale into fp8 range

F32 = mybir.dt.float32
BF16 = mybir.dt.bfloat16
F8 = mybir.dt.float8e4
EXP = mybir.ActivationFunctionType.Exp
ALU = mybir.AluOpType

LAST_RESULT = None

_CACHE = {}


def _build():
    key = 0
    if key in _CACHE:
        return _CACHE[key]

    nc = bacc.Bacc("TRN2", target_bir_lowering=False, debug=False,
                   num_devices=NCORES)

    stem_d = nc.dram_tensor("stem", [B, DLOC, N], BF16, kind="ExternalInput").ap()
    vocab_d = nc.dram_tensor("vocab", [V, DLOC, N], F8, kind="ExternalInput").ap()
    mor_d = nc.dram_tensor("morpho", [B, DM], BF16, kind="ExternalInput").ap()
    waff_d = nc.dram_tensor("waffix", [DM, V], BF16, kind="ExternalInput").ap()
    pv_d = nc.dram_tensor("pivot", [2, 2, 128, 5, N], BF16, kind="ExternalInput").ap()
    abf_d = nc.dram_tensor("abf", [1, 9], F32, kind="ExternalInput").ap()
    out_d = nc.dram_tensor("out", [B, DLOC, N], BF16, kind="ExternalOutput").ap()

    from contextlib import ExitStack

    with tile.TileContext(nc) as tc, ExitStack() as ctx:
        const = ctx.enter_context(tc.tile_pool(name="const", bufs=1))

        identf = const.tile([128, 128], F32)
        make_identity(nc, identf[:, :])
        identb = const.tile([128, 128], BF16)
        nc.scalar.copy(identb[:, :], identf[:, :])

        attnT = const.tile([128, 4, B], F8)        # [v_part, vc, b], 64*attn
        wc_sb = const.tile([128, BCH, N], BF16)    # [b_part, cb, n]
        w_bcast = const.tile([128, 20], F32)
        wsb = const.tile([128, V], BF16)           # W_affix resident [dm, v]
        mor_all = const.tile([128, BCH, DM], BF16)

        # ---------- phase P: mixture weights + pivots + AllGather ----------
        small = ctx.enter_context(tc.tile_pool(name="small", bufs=1))
        pvp = tc.alloc_tile_pool(name="pv", bufs=1)
        pv = pvp.tile([128, 4, 5, N], BF16)
        abf = small.tile([1, 9], F32)
        nc.sync.dma_start(abf[0:1, :], abf_d[:, :])
        for ij in range(4):
            i, j = divmod(ij, 2)
            nc.sync.dma_start(pv[:, ij, :, :], pv_d[i, j, :, :, :])
        pvE = pvp.tile([128, 4, 5, N], BF16)
        sP = pvp.tile([128, 4, 5], F32)
        # exp+rowsum pipelined per (i,j) chunk so compute starts on the
        # first pivot DMA rather than the last
        for ij in range(4):
            nc.scalar.activation(
                pvE[:, ij, :, :].rearrange("p k n -> p (k n)"),
                pv[:, ij, :, :].rearrange("p k n -> p (k n)"), EXP,
            )
            nc.vector.reduce_sum(sP[:, ij, :], pvE[:, ij, :, :],
                                 axis=mybir.AxisListType.X)

        # mixture weights w20[g] = a_i * b_j * f_k (g = (i*2+j)*5+k)
        eabf = small.tile([1, 9], F32)
        sums3 = small.tile([1, 3], F32)
        nc.scalar.activation(eabf[0:1, 0:2], abf[0:1, 0:2], EXP, accum_out=sums3[0:1, 0:1])
        nc.scalar.activation(eabf[0:1, 2:4], abf[0:1, 2:4], EXP, accum_out=sums3[0:1, 1:2])
        nc.scalar.activation(eabf[0:1, 4:9], abf[0:1, 4:9], EXP, accum_out=sums3[0:1, 2:3])
        rsum = small.tile([1, 3], F32)
        nc.vector.reciprocal(rsum[0:1, :], sums3[0:1, :])
        t4 = small.tile([1, 4], F32)
        nc.vector.tensor_mul(
            t4[0:1, :].rearrange("p (i j) -> p i j", i=2),
            eabf[0:1, 0:2].rearrange("p (i j) -> p i j", j=1).to_broadcast((1, 2, 2)),
            eabf[0:1, 2:4].rearrange("p (i j) -> p i j", i=1).to_broadcast((1, 2, 2)),
        )
        t20 = small.tile([1, 20], F32)
        nc.vector.tensor_mul(
            t20[0:1, :].rearrange("p (g k) -> p g k", g=4),
            t4[0:1, :].rearrange("p (g k) -> p g k", k=1).to_broadcast((1, 4, 5)),
            eabf[0:1, 4:9].rearrange("p (g k) -> p g k", g=1).to_broadcast((1, 4, 5)),
        )
        rr = small.tile([1, 1], F32)
        nc.vector.tensor_mul(rr[0:1, :], rsum[0:1, 0:1], rsum[0:1, 1:2])
        rrr = small.tile([1, 1], F32)
        nc.vector.tensor_mul(rrr[0:1, :], rr[0:1, :], rsum[0:1, 2:3])
        w20 = small.tile([1, 20], F32)
        nc.vector.tensor_scalar_mul(w20[0:1, :], t20[0:1, :], rrr[0:1, 0:1])
        nc.gpsimd.partition_broadcast(w_bcast[:, :], w20[0:1, :])

        # weighted sum over the 20 groups, then cumsum. Few big DVE ops
        # beat a per-chunk pipeline here: this chain sits at the head of
        # the DVE queue, and many small sem-gated ops there delay the
        # main-loop runahead subs queued behind them.
        rP = pvp.tile([128, 20], F32)
        nc.vector.reciprocal(rP[:, :], sP[:, :, :].rearrange("p i k -> p (i k)"))
        rPw = pvp.tile([128, 20], BF16)
        nc.vector.tensor_mul(rPw[:, :], rP[:, :], w_bcast[:, :])
        pvS = pvp.tile([128, 20, N], BF16)
        nc.vector.tensor_mul(
            pvS[:, :, :],
            pvE[:, :, :, :].rearrange("p i k n -> p (i k) n"),
            rPw[:, :].rearrange("p (g o) -> p g o", o=1).to_broadcast((128, 20, N)),
        )
        t10 = pvp.tile([128, 10, N], BF16)
        nc.vector.tensor_add(t10[:, :, :], pvS[:, 0:10, :], pvS[:, 10:20, :])
        t5 = pvp.tile([128, 5, N], BF16)
        nc.vector.tensor_add(t5[:, :, :], t10[:, 0:5, :], t10[:, 5:10, :])
        t2 = pvp.tile([128, 2, N], BF16)
        nc.vector.tensor_add(t2[:, :, :], t5[:, 0:2, :], t5[:, 2:4, :])
        acc1 = pvp.tile([128, N], BF16)
        nc.vector.tensor_add(acc1[:, :], t2[:, 0, :], t2[:, 1, :])
        accf = pvp.tile([128, N], F32)
        nc.vector.tensor_add(accf[:, :], acc1[:, :], t5[:, 4, :])
        wCl = pvp.tile([128, N], BF16)
        nc.vector.tensor_tensor_scan(
            wCl[:, :], data0=accf[:, :], data1=accf[:, :], initial=0.0,
            op0=ALU.add, op1=ALU.bypass,
        )
        dram = ctx.enter_context(tc.tile_pool(name="dram", bufs=1, space="DRAM"))
        wc_in = dram.tile([128, N], BF16)
        wc_out = nc.dram_tensor("wc_gath", [B, N], BF16, addr_space="Shared").ap()
        nc.sync.dma_start(wc_in[:, :], wCl[:, :])
        nc.gpsimd.collective_compute(
            "AllGather", ALU.bypass,
            replica_groups=[list(range(NCORES))],
            ins=[wc_in[:, :].opt()], outs=[wc_out[:, :].opt()],
        )
        nc.sync.dma_start(
            wc_sb[:, :, :],
            wc_out[:, :].rearrange("(c p) n -> p c n", p=128),
        )
        pvp.release()

        # ---------- phase A: attention -> fp8 attnT ----------
        nc.sync.dma_start(wsb[:, :], waff_d[:, :])
        nc.sync.dma_start(
            mor_all[:, :, :],
            mor_d[:, :].rearrange("(c p) m -> p c m", p=128),
        )
        bp = tc.alloc_tile_pool(name="attn", bufs=2)
        psB = tc.alloc_tile_pool(name="psB", bufs=2, space="PSUM")
        psT = tc.alloc_tile_pool(name="psT", bufs=2, space="PSUM")

        for cb in range(BCH):
            morT_ps = psB.tile([128, DM], BF16, tag="morT_ps", name=f"mtp{cb}")
            nc.tensor.transpose(morT_ps[:, :], mor_all[:, cb, :], identb[:, :])
            morT = bp.tile([128, DM], BF16, tag="morT", name=f"mt{cb}")
            nc.vector.tensor_copy(morT[:, :], morT_ps[:, :])
            lg_ps = psB.tile([128, V], F32, tag="lg_ps", name=f"lgp{cb}")
            nc.tensor.matmul(lg_ps[:, :], lhsT=morT[:, :], rhs=wsb[:, :],
                             start=True, stop=True)
            E = bp.tile([128, V], BF16, tag="E", name=f"E{cb}")
            sE = bp.tile([128, 1], F32, tag="sE", name=f"sE{cb}")
            nc.scalar.activation(E[:, :], lg_ps[:, :], EXP, accum_out=sE[:, :])
            rE = bp.tile([128, 1], F32, tag="rE", name=f"rE{cb}")
            nc.vector.reciprocal(rE[:, :], sE[:, :])
            rE64 = bp.tile([128, 1], F32, tag="rE64", name=f"rE64{cb}")
            nc.vector.tensor_scalar_mul(rE64[:, :], rE[:, :], ASCALE)
            attnb = bp.tile([128, V], BF16, tag="atb", name=f"atb{cb}")
            nc.scalar.mul(attnb[:, :], E[:, :], rE64[:, 0:1])
            tpb = psT.tile([128, V], BF16, tag="tpb", name=f"tpb{cb}")
            for vc in range(4):
                nc.tensor.transpose(tpb[:, ts(vc, 128)], attnb[:, ts(vc, 128)],
                                    identb[:, :])
            nc.scalar.copy(
                attnT[:, :, ts(cb, 128)],
                tpb[:, :].rearrange("p (c b) -> p c b", c=4),
            )
        psT.release()
        psB.release()
        bp.release()

        # ---------- phase D: main loop ----------
        stp = ctx.enter_context(tc.tile_pool(name="stem", bufs=7))
        vqp = ctx.enter_context(tc.tile_pool(name="vq", bufs=2))
        afp = ctx.enter_context(tc.tile_pool(name="affx", bufs=2))
        dlp = ctx.enter_context(tc.tile_pool(name="delta", bufs=8))
        prp = ctx.enter_context(tc.tile_pool(name="prod", bufs=1))
        otp = ctx.enter_context(tc.tile_pool(name="outp", bufs=2))
        psD = ctx.enter_context(tc.tile_pool(name="psD", bufs=2, space="PSUM"))

        for r in range(2):
            vq = vqp.tile([128, 4, HALF], F8)
            for vc in range(4):
                nc.sync.dma_start(
                    vq[:, vc, :],
                    vocab_d[ts(vc, 128), ts(r, DHALF), :].rearrange("p d n -> p (d n)"),
                )
            for cb in range(BCH):
                stem_t = stp.tile([128, HALF], BF16)
                nc.sync.dma_start(
                    stem_t[:, :],
                    stem_d[ts(cb, 128), ts(r, DHALF), :].rearrange("p d n -> p (d n)"),
                )
                affx = afp.tile([128, HALF], BF16)
                for h in range(NH):
                    ps = psD.tile([128, PSW], F32)
                    # [vc outer, t inner]: 4 consecutive matmuls share the
                    # same stationary tile so the LDW dedup drops reloads
                    for vc in range(4):
                        for t in range(PSW // 512):
                            col = h * PSW + t * 512
                            nc.tensor.matmul(
                                ps[:, ts(t, 512)],
                                lhsT=attnT[:, vc, ts(cb, 128)],
                                rhs=vq[:, vc, col:col + 512],
                                start=(vc == 0), stop=(vc == 3),
                            )
                    nc.scalar.mul(affx[:, ts(h, PSW)], ps[:, :], 1.0 / ASCALE)
                delta = dlp.tile([128, HALF], BF16)
                prod = prp.tile([128, HALF], BF16)
                out_t = otp.tile([128, HALF], BF16)
                nc.vector.tensor_sub(delta[:, :], affx[:, :], stem_t[:, :])
                nc.vector.tensor_mul(
                    prod[:, :].rearrange("p (a n) -> p a n", n=N),
                    delta[:, :].rearrange("p (a n) -> p a n", n=N),
                    wc_sb[:, cb:cb + 1, :].to_broadcast((128, HALF // N, N)),
                )
                nc.vector.tensor_add(out_t[:, :], prod[:, :], stem_t[:, :])
                nc.sync.dma_start(
                    out_d[ts(cb, 128), ts(r, DHALF), :].rearrange("p d n -> p (d n)"),
                    out_t[:, :],
                )

    nc.compile()
    _dedup_ldweights(nc)
    _CACHE[key] = nc
    return nc


def kernel(stem_form, morphosyn, pivot_logits, W_affix, affix_vocab,
           alpha, beta, phi, max_len):
    global LAST_RESULT
    abf = np.concatenate([
        np.asarray(alpha, np.float32).ravel(),
        np.asarray(beta, np.float32).ravel(),
        np.asarray(phi, np.float32).ravel(),
    ]).reshape(1, 9)

    nc = _build()

    stem_np = np.asarray(stem_form, np.float32).astype(ml_dtypes.bfloat16)
    vocab_np = np.asarray(affix_vocab, np.float32).astype(ml_dtypes.float8_e4m3)
    mor_np = np.asarray(morphosyn, np.float32).astype(ml_dtypes.bfloat16)
    waff_np = np.asarray(W_affix, np.float32).astype(ml_dtypes.bfloat16)
    pv_np = np.asarray(pivot_logits, np.float32).astype(ml_dtypes.bfloat16)

    in_maps = []
    for c in range(NCORES):
        dlo, dhi = c * DLOC, (c + 1) * DLOC
        in_maps.append({
            "stem": np.ascontiguousarray(stem_np[:, dlo:dhi, :]),
            "vocab": np.ascontiguousarray(vocab_np[:, dlo:dhi, :]),
            "morpho": mor_np,
            "waffix": waff_np,
            "pivot": np.ascontiguousarray(pv_np[:, :, c * 128:(c + 1) * 128, :, :]),
            "abf": abf,
        })

    LAST_RESULT = run_bass_kernel_spmd(nc, in_maps, core_ids=list(range(NCORES)))
    outs = [LAST_RESULT.results[c]["out"] for c in range(NCORES)]
    out = np.concatenate([o.astype(np.float32) for o in outs], axis=1)
    return np.ascontiguousarray(out)
